# revision 2
# baseline (speedup 1.0000x reference)
"""Trainium2 Bass kernel v2 for nn_CrossAttention (8-core data-parallel over batch).

Math (per batch b = one NeuronCore):
  x1 = x + PEx ; y1 = y + PEy           (raw-reshape positional encodings)
  q  = conv3x3(relu(conv3x3(x1,wq1)+bq1), wq2)+bq2   viewed as (1024,128)
  k  = conv3x3(relu(conv3x3(y1,wk1)+bk1), wk2)+bk2   viewed as (4096,128)
  out = softmax(s * q @ k.T) @ z.flat                (s = 1/sqrt(128))

v2 design vs v1:
  - everything bf16 on the wires: inputs/PE tables/weights DMA'd as bf16
    (halves HBM traffic, kills the f32r weight-convert pass)
  - convs as 9 accumulating bf16 matmuls (1 cyc/col, same as f32r, but
    enables bf16 transposes at 1 cyc and 2x DVE copies)
  - fixed per-row softmax shift from the stride-32 key subsample of chunk 0
    (measured gap to true rowmax < 34 << 88 -> exact after renorm); removes
    the online max/rescale machinery entirely
  - conv epilogues (bias+relu / bias+scale) on Pool; exp on Act with fused
    bias + denominator accumulation; numerator via DVE bf16 stt at 2x
  - PE emission interleaves conv matmuls between attention logits so the PE
    never waits on Act's exp; psl double-buffered in PSUM (2+2+4 banks)
"""

import numpy as np
import ml_dtypes

import concourse.bass as bass
import concourse.mybir as mybir
import concourse.tile as tile
from concourse import bacc
from concourse.bass import ts
from concourse.bass_utils import run_bass_kernel_spmd

F32 = mybir.dt.float32
BF16 = mybir.dt.bfloat16
AF = mybir.ActivationFunctionType
ALU = mybir.AluOpType

C = 128
A = 32          # q spatial side
H = 64          # k spatial side
SQ = A * A      # 1024
SK = H * H      # 4096
SCALE = float(C ** -0.5)
N_CORES = 8
NPBF = ml_dtypes.bfloat16


def _make_pe(dim, length):
    pos = np.arange(length, dtype=np.float32)[:, None]
    div = np.exp(np.arange(0, dim, 2, dtype=np.float32) * np.float32(-np.log(10000.0) / dim))
    pe = np.zeros((length, dim), dtype=np.float32)
    pe[:, 0::2] = np.sin(pos * div)
    pe[:, 1::2] = np.cos(pos * div)
    return pe


def _build_program(repeat=1, staggered=False):
    nc = bacc.Bacc("TRN2", target_bir_lowering=False, debug=False, num_devices=N_CORES)

    dx = nc.dram_tensor("x", [C, SQ], BF16, kind="ExternalInput")
    dy = nc.dram_tensor("y", [C, SK], BF16, kind="ExternalInput")
    dv = nc.dram_tensor("vz", [1, SK], BF16, kind="ExternalInput")
    dw = {n: nc.dram_tensor(n, [C, 9 * C], BF16, kind="ExternalInput")
          for n in ("wq1", "wq2", "wk1", "wk2")}
    db = {n: nc.dram_tensor(n, [C, 1], F32, kind="ExternalInput")
          for n in ("bq1", "bq2", "bk1", "bk2")}
    dpex = nc.dram_tensor("pex", [C, SQ], BF16, kind="ExternalInput")
    dpey = nc.dram_tensor("pey", [C, SK], BF16, kind="ExternalInput")
    dident = nc.dram_tensor("ident", [C, C], BF16, kind="ExternalInput")
    dout = nc.dram_tensor("out", [SQ, 1], F32, kind="ExternalOutput")

    XP, YP = A + 2, H + 2          # padded sides: 34, 66
    with tile.TileContext(nc) as tc:
        with (
            tc.tile_pool(name="const", bufs=1) as cst,
            tc.tile_pool(name="kimg", bufs=3) as kip,
            tc.tile_pool(name="pp", bufs=3) as ppool,
            tc.tile_pool(name="scr", bufs=3) as scrp,
            tc.tile_pool(name="psc", bufs=2, space="PSUM") as psc,
            tc.tile_pool(name="pst", bufs=2, space="PSUM") as pst,
            tc.tile_pool(name="psa", bufs=2, space="PSUM") as psa,
        ):
          import contextlib
          loop_cm = (tc.For_i(0, repeat, 1,
                              hint_engines=(mybir.EngineType.PE, mybir.EngineType.Activation,
                                            mybir.EngineType.DVE, mybir.EngineType.SP,
                                            mybir.EngineType.Pool),
                              staggered_reset=staggered)
                     if repeat > 1 else contextlib.nullcontext())
          # ---- once-only prologue (outside the repeat loop): PE warm-up
          # ramps the pstate while the first DMAs land; Exp table preload;
          # constant zero borders of the padded conv buffers ----
          wmem = cst.tile([C, 512], BF16, tag="wmem")
          nc.gpsimd.memset(wmem[:], 0.0)
          wps = psa.tile([C, 1024], F32, tag="psl", name="wps")
          for i in range(8):
              nc.tensor.matmul(wps[:, 0:512], wmem[:, 0:C], wmem[:],
                               start=True, stop=True)
          wexp = cst.tile([C, 1], BF16, tag="wexp")
          nc.scalar.activation(wexp[:], wmem[:, 0:1], AF.Exp)

          zrow = cst.tile([C, YP], BF16, tag="zrow")
          nc.gpsimd.memset(zrow[:], 0.0)

          def pad_tile(tag, side):
              t = cst.tile([C, side * side], BF16, tag=tag, name=tag)
              t3 = t[:].rearrange("p (r c) -> p r c", c=side)
              zr = zrow[:, 0:side].rearrange("p (a c) -> p a c", a=1)
              zc = zrow[:, 0:side - 2].rearrange("p (r a) -> p r a", a=1)
              nc.gpsimd.tensor_copy(t3[:, 0:1, :], zr)
              nc.gpsimd.tensor_copy(t3[:, side - 1:side, :], zr)
              nc.gpsimd.tensor_copy(t3[:, 1:side - 1, 0:1], zc)
              nc.gpsimd.tensor_copy(t3[:, 1:side - 1, side - 1:side], zc)
              return t

          x_pad = pad_tile("x_pad", XP)
          t1q = pad_tile("t1q", XP)
          y_pad = pad_tile("y_pad", YP)
          t1k = pad_tile("t1k", YP)

          with loop_cm:
            # ---- DMA loads (priority order: the exact q-path critical chain
            # first: x half 0, wq1, pex half 0, then the rest) ----
            x_raw = cst.tile([C, SQ], BF16, tag="x_raw")
            pex = cst.tile([C, SQ], BF16, tag="pex")
            w_sb, b_sb = {}, {}

            def load_w(n):
                w_sb[n] = cst.tile([C, 9 * C], BF16, tag=n, name=n + "_sb")
                for h in range(2):
                    nc.sync.dma_start(out=w_sb[n][:, ts(h, 576)], in_=dw[n].ap()[:, ts(h, 576)])

            def load_b(n):
                b_sb[n] = cst.tile([C, 1], F32, tag=n, name=n + "_sb")
                nc.sync.dma_start(out=b_sb[n][:], in_=db[n].ap())

            nc.sync.dma_start(out=x_raw[:, ts(0, SQ // 2)], in_=dx.ap()[:, ts(0, SQ // 2)])
            load_w("wq1")
            nc.sync.dma_start(out=pex[:, ts(0, SQ // 2)], in_=dpex.ap()[:, ts(0, SQ // 2)])
            load_b("bq1")
            nc.sync.dma_start(out=x_raw[:, ts(1, SQ // 2)], in_=dx.ap()[:, ts(1, SQ // 2)])
            nc.sync.dma_start(out=pex[:, ts(1, SQ // 2)], in_=dpex.ap()[:, ts(1, SQ // 2)])

            y_raw = cst.tile([C, SK], BF16, tag="y_raw")
            pey = cst.tile([C, SK], BF16, tag="pey")
            v_rep = cst.tile([C, SK], BF16, tag="v_rep")

            def load_y(h):
                nc.sync.dma_start(out=y_raw[:, ts(h, SK // 4)], in_=dy.ap()[:, ts(h, SK // 4)])
                nc.sync.dma_start(out=pey[:, ts(h, SK // 4)], in_=dpey.ap()[:, ts(h, SK // 4)])

            def load_v(h):
                nc.sync.dma_start(out=v_rep[:, ts(h, SK // 4)],
                                  in_=dv.ap()[:, ts(h, SK // 4)].broadcast_to((C, SK // 4)))

            load_y(0)
            load_w("wk1"); load_b("bk1")
            load_w("wq2"); load_b("bq2")
            ident = cst.tile([C, C], BF16, tag="ident")
            nc.sync.dma_start(out=ident[:], in_=dident.ap())
            load_y(1)
            load_w("wk2"); load_b("bk2")
            load_y(2)
            load_v(0)
            load_y(3)
            load_v(1); load_v(2); load_v(3)

            x_pad3 = x_pad[:].rearrange("p (r c) -> p r c", c=XP)
            t1q3 = t1q[:].rearrange("p (r c) -> p r c", c=XP)
            y_pad3 = y_pad[:].rearrange("p (r c) -> p r c", c=YP)
            t1k3 = t1k[:].rearrange("p (r c) -> p r c", c=YP)

            # x1 = x + PEx into padded interior (DVE: Pool tensor_tensor is 2x
            # slower and sits behind the pad ops in the Pool queue); split in
            # halves so the first conv taps start as soon as piece 0 lands
            for h in range(2):
                nc.vector.tensor_tensor(
                    out=x_pad3[:, 16 * h + 1:16 * h + 17, 1:A + 1],
                    in0=x_raw[:, ts(h, SQ // 2)].rearrange("p (r c) -> p r c", c=A),
                    in1=pex[:, ts(h, SQ // 2)].rearrange("p (r c) -> p r c", c=A),
                    op=ALU.add)
            # y1 = y + PEy in quarters (16 image rows each); DVE, not Pool:
            # Pool tensor_tensor is 2x slower and would block conv epilogues
            for h in range(4):
                nc.vector.tensor_tensor(
                    out=y_pad3[:, 16 * h + 1:16 * h + 17, 1:H + 1],
                    in0=y_raw[:, ts(h, SK // 4)].rearrange("p (r c) -> p r c", c=H),
                    in1=pey[:, ts(h, SK // 4)].rearrange("p (r c) -> p r c", c=H),
                    op=ALU.add)

            def conv_mms(src3, w, rows0, nrows, side_c, ps):
                """Emit the 9 accumulating bf16 matmuls for one conv tile."""
                ps3 = ps[:].rearrange("p (r c) -> p r c", c=side_c)
                i = 0
                for dyy in range(3):
                    for dxx in range(3):
                        rhs = src3[:, rows0 + dyy: rows0 + dyy + nrows,
                                   dxx: dxx + side_c]
                        nc.tensor.matmul(ps3, w[:, ts(i, C)], rhs,
                                         start=(i == 0), stop=(i == 8))
                        i += 1

            # ---- q path (as closures; interleaved with early k tiles) ----
            q_img = cst.tile([C, SQ], BF16, tag="q_img")
            qT = cst.tile([C, SQ], BF16, tag="qT")

            def q_conv1(n):
                ps1 = psc.tile([C, 512], F32, tag="cps", name=f"qps1_{n}")
                conv_mms(x_pad3, w_sb["wq1"], 16 * n, 16, A, ps1)
                nc.scalar.activation(
                    t1q3[:, 16 * n + 1:16 * n + 17, 1:A + 1],
                    ps1[:].rearrange("p (r c) -> p r c", c=A),
                    AF.Relu, bias=b_sb["bq1"][:])

            def q_conv2(n):
                ps2 = psc.tile([C, 512], F32, tag="cps", name=f"qps2_{n}")
                conv_mms(t1q3, w_sb["wq2"], 16 * n, 16, A, ps2)
                # fold the attention scale into q here
                # host sends bq2 pre-multiplied by SCALE: (p + b)*s = p*s + b*s
                nc.scalar.activation(
                    q_img[:, ts(n, 512)], ps2[:],
                    AF.Identity, bias=b_sb["bq2"][:], scale=SCALE)

            def q_transpose(g):
                pt = pst.tile([C, 512], BF16, tag="tps", name=f"ptq_{g}")
                for i in range(4):
                    nc.tensor.transpose(pt[:, ts(i, C)], q_img[:, ts(4 * g + i, C)], ident[:])
                nc.vector.tensor_copy(qT[:, ts(g, 512)], pt[:])

            # ---- k path ops (as closure lists for interleaving) ----
            kT = cst.tile([C, SK], BF16, tag="kT")

            def conv1_k_ops(t):
                ops = []
                ps_box = []
                def mk(i0, i1):
                    def f():
                        if not ps_box:
                            ps_box.append(psc.tile([C, 512], F32, tag="cps", name=f"cps_{t}"))
                        ps3 = ps_box[0][:].rearrange("p (r c) -> p r c", c=H)
                        j = 0
                        for dyy in range(3):
                            for dxx in range(3):
                                if i0 <= j < i1:
                                    rhs = y_pad3[:, 8 * t + dyy: 8 * t + dyy + 8,
                                                 dxx: dxx + H]
                                    nc.tensor.matmul(ps3, w_sb["wk1"][:, ts(j, C)], rhs,
                                                     start=(j == 0), stop=(j == 8))
                                j += 1
                    return f
                for (i0, i1) in ((0, 3), (3, 6), (6, 9)):
                    ops.append(mk(i0, i1))
                def epi():
                    if t <= 2:   # prefix: Act is idle before the exp stream
                        nc.scalar.activation(
                            t1k3[:, 8 * t + 1:8 * t + 9, 1:H + 1],
                            ps_box[0][:].rearrange("p (r c) -> p r c", c=H),
                            AF.Relu, bias=b_sb["bk1"][:])
                    else:
                        nc.vector.tensor_scalar(
                            out=t1k3[:, 8 * t + 1:8 * t + 9, 1:H + 1],
                            in0=ps_box[0][:].rearrange("p (r c) -> p r c", c=H),
                            scalar1=b_sb["bk1"][:], scalar2=0.0, op0=ALU.add, op1=ALU.max)
                ops.append(epi)
                return ops

            def conv2_k_ops(t):
                ops = []
                ps_box = []
                kimg_box = []
                def mk(i0, i1):
                    def f():
                        if not ps_box:
                            ps_box.append(psc.tile([C, 512], F32, tag="cps", name=f"cps_{t}"))
                        ps3 = ps_box[0][:].rearrange("p (r c) -> p r c", c=H)
                        j = 0
                        for dyy in range(3):
                            for dxx in range(3):
                                if i0 <= j < i1:
                                    rhs = t1k3[:, 8 * t + dyy: 8 * t + dyy + 8,
                                               dxx: dxx + H]
                                    nc.tensor.matmul(ps3, w_sb["wk2"][:, ts(j, C)], rhs,
                                                     start=(j == 0), stop=(j == 8))
                                j += 1
                    return f
                for (i0, i1) in ((0, 3), (3, 6), (6, 9)):
                    ops.append(mk(i0, i1))
                def epi():
                    kimg_box.append(kip.tile([C, 512], BF16, tag="kimg", name=f"kimg_{t}"))
                    if t <= 1:   # prefix: Act is idle before the exp stream
                        nc.scalar.activation(
                            kimg_box[0][:], ps_box[0][:],
                            AF.Identity, bias=b_sb["bk2"][:])
                    else:
                        nc.vector.tensor_scalar(
                            out=kimg_box[0][:], in0=ps_box[0][:],
                            scalar1=b_sb["bk2"][:], scalar2=None, op0=ALU.add)
                ops.append(epi)
                def trcopy():
                    pt = pst.tile([C, 512], BF16, tag="tps", name=f"ptk_{t}")
                    for i in range(4):
                        nc.tensor.transpose(pt[:, ts(i, C)], kimg_box[0][:, ts(i, C)],
                                            ident[:])
                    if t >= 2:   # balance: Act has slack here, DVE paces chunks
                        nc.scalar.activation(kT[:, ts(t, 512)], pt[:], AF.Identity)
                    else:
                        nc.vector.tensor_copy(kT[:, ts(t, 512)], pt[:])
                ops.append(trcopy)
                return ops

            # ---- attention state ----
            negM = cst.tile([C, 8], F32, tag="negM")
            dacc = cst.tile([C, 32], F32, tag="dacc")   # col = m*4 + c
            nacc = cst.tile([C, 32], F32, tag="nacc")

            # per-row shifts from the stride-32 key subsample (= kT cols
            # 0:128): tiny matmuls + batched reduces, split in two halves so
            # the first half (m=0..3, all chunk 0 needs to start) comes as
            # early as possible; the second half rides in chunk-0 filler.
            prepass_box = {}

            def negmax_prepass_mms(half):
                sub = psc.tile([C, 512], F32, tag="cps", name=f"pslsub{half}")
                for j in range(4):
                    m = 4 * half + j
                    nc.tensor.matmul(sub[:, ts(j, C)], qT[:, ts(m, C)],
                                     kT[:, 0:C], start=True, stop=True)
                prepass_box[half] = sub

            def negmax_prepass_reduce(half):
                sub = prepass_box[half]
                if half == 0:
                    # m=0 alone first: it gates exp(0,0)
                    nc.vector.tensor_reduce(
                        out=negM[:, 0:1], in_=sub[:, 0:C],
                        axis=mybir.AxisListType.X, op=ALU.max, negate=True)
                    nc.vector.tensor_reduce(
                        out=negM[:, 1:4],
                        in_=sub[:, C:].rearrange("p (m k) -> p m k", k=C),
                        axis=mybir.AxisListType.X, op=ALU.max, negate=True)
                else:
                    nc.vector.tensor_reduce(
                        out=negM[:, 4:8],
                        in_=sub[:].rearrange("p (m k) -> p m k", k=C),
                        axis=mybir.AxisListType.X, op=ALU.max, negate=True)

            def att_ops(c, m):
                psl = psa.tile([C, 1024], F32, tag="psl", name=f"psl_{c}_{m}")
                for u in range(2):   # one matmul cannot span two PSUM banks
                    nc.tensor.matmul(psl[:, ts(u, 512)], qT[:, ts(m, C)],
                                     kT[:, 1024 * c + 512 * u: 1024 * c + 512 * (u + 1)],
                                     start=True, stop=True)
                P = ppool.tile([C, 1024], BF16, tag="P", name=f"P_{c}_{m}")
                col = 4 * m + c
                nc.scalar.activation(P[:], psl[:], AF.Exp, bias=negM[:, m:m + 1],
                                     scale=1.0, accum_out=dacc[:, col:col + 1])
                scrap = scrp.tile([C, 1024], BF16, tag="scrap", name=f"scrap_{c}_{m}")
                nc.vector.scalar_tensor_tensor(out=scrap[:], in0=P[:], scalar=1.0,
                                               in1=v_rep[:, ts(c, 1024)],
                                               op0=ALU.bypass, op1=ALU.mult,
                                               accum_out=nacc[:, col:col + 1])

            # ---- emission ----
            # conv1 must run one tile ahead of conv2 (conv2(t) reads t1k rows
            # up to 8t+9, written by conv1(t+1)).  The q path is interleaved
            # with the early k tiles so the first attention chunk (and with it
            # the Act exp pipeline, the closing constraint) starts as early as
            # possible.
            q_conv1(0)
            q_conv1(1)
            for op in conv1_k_ops(0):
                op()
            q_conv2(0)
            q_transpose(0)
            for op in conv1_k_ops(1):
                op()
            for op in conv2_k_ops(0):
                op()
            negmax_prepass_mms(0)
            for op in conv1_k_ops(2):
                op()
            for op in conv2_k_ops(1):
                op()
            negmax_prepass_reduce(0)

            chunk_convs = {
                0: [("c1", 3), ("c2", 2), ("c1", 4), ("c2", 3)],
                1: [("c1", 5), ("c2", 4), ("c1", 6), ("c2", 5)],
                2: [("c1", 7), ("c2", 6), ("c2", 7)],
                3: [],
            }
            for cchunk in range(4):
                conv_q = []
                for kind, t in chunk_convs[cchunk]:
                    conv_q.extend(conv1_k_ops(t) if kind == "c1" else conv2_k_ops(t))
                if cchunk == 0:
                    # deferred q half (m-blocks 4..7, first needed at m=4),
                    # interleaved into the conv closure stream
                    conv_q.insert(2, lambda: q_conv2(1))
                    conv_q.insert(5, lambda: q_transpose(1))
                    conv_q.insert(6, lambda: negmax_prepass_mms(1))
                    conv_q.insert(7, lambda: negmax_prepass_reduce(1))
                n_ops = len(conv_q)
                for m in range(8):
                    att_ops(cchunk, m)
                    # interleave conv work after each logits group, front-
                    # loaded into m=0..5 so the next chunk's kT (trcopy) is
                    # ready before its first logits
                    k0 = min(n_ops, (n_ops * m) // 6)
                    k1 = min(n_ops, (n_ops * (m + 1)) // 6)
                    for op in conv_q[k0:k1]:
                        op()

            # ---- finale ----
            dsum = cst.tile([C, 8], F32, tag="dsum")
            nsum = cst.tile([C, 8], F32, tag="nsum")
            recip = cst.tile([C, 8], F32, tag="recip")
            res = cst.tile([C, 8], F32, tag="res")
            nc.vector.tensor_reduce(out=dsum[:], in_=dacc[:].rearrange("p (m c) -> p m c", c=4),
                                    axis=mybir.AxisListType.X, op=ALU.add)
            nc.vector.tensor_reduce(out=nsum[:], in_=nacc[:].rearrange("p (m c) -> p m c", c=4),
                                    axis=mybir.AxisListType.X, op=ALU.add)
            nc.vector.reciprocal(recip[:], dsum[:])
            nc.vector.tensor_tensor(out=res[:], in0=nsum[:], in1=recip[:], op=ALU.mult)
            nc.sync.dma_start(out=dout.ap().rearrange("(co m) one -> co (m one)", m=8),
                              in_=res[:])

    nc.compile()
    return nc


_NC_CACHE = []


def _host_prep(x, y, z, wq1, bq1, wq2, bq2, wk1, bk1, wk2, bk2):
    B = x.shape[0]
    wmap = {}
    for name, w in (("wq1", wq1), ("wq2", wq2), ("wk1", wk1), ("wk2", wk2)):
        wmap[name] = np.ascontiguousarray(
            np.asarray(w, dtype=np.float32).transpose(1, 2, 3, 0).reshape(C, 9 * C)
        ).astype(NPBF)
    bmap = {"bq1": bq1, "bq2": np.asarray(bq2, np.float32) * SCALE,
            "bk1": bk1, "bk2": bk2}
    bmap = {n: np.ascontiguousarray(np.asarray(b, dtype=np.float32).reshape(C, 1))
            for n, b in bmap.items()}
    pex = np.ascontiguousarray(_make_pe(C, SQ).reshape(C, SQ)).astype(NPBF)
    pey = np.ascontiguousarray(_make_pe(C, SK).reshape(C, SK)).astype(NPBF)
    ident = np.eye(C, dtype=np.float32).astype(NPBF)
    # v in kT column order: col m*128+co  ->  z_flat[co*32+m]
    zperm = np.ascontiguousarray(
        np.asarray(z, np.float32).reshape(B, SK).reshape(B, C, SK // C)
        .transpose(0, 2, 1).reshape(B, 1, SK)).astype(NPBF)

    in_maps = []
    for b in range(B):
        m = {
            "x": np.ascontiguousarray(np.asarray(x, np.float32)[b].reshape(C, SQ)).astype(NPBF),
            "y": np.ascontiguousarray(np.asarray(y, np.float32)[b].reshape(C, SK)).astype(NPBF),
            "vz": zperm[b],
            "pex": pex, "pey": pey, "ident": ident,
        }
        m.update(wmap)
        m.update(bmap)
        in_maps.append(m)
    return in_maps


def kernel(x, y, z, wq1, bq1, wq2, bq2, wk1, bk1, wk2, bk2):
    x = np.asarray(x, dtype=np.float32)
    B = x.shape[0]
    assert B == N_CORES

    if not _NC_CACHE:
        _NC_CACHE.append(_build_program())
    nc = _NC_CACHE[0]

    in_maps = _host_prep(x, y, z, wq1, bq1, wq2, bq2, wk1, bk1, wk2, bk2)
    res = run_bass_kernel_spmd(nc, in_maps, core_ids=list(range(N_CORES)))
    out = np.stack([res.results[b]["out"].reshape(SQ, 1) for b in range(B)])
    return out.astype(np.float32)
